# revision 25
# baseline (speedup 1.0000x reference)
"""Trainium2 Bass kernel: pre-LN single-head causal attention + residual.

Reference computation (B=4, S=2048, H=K=2048, fp32):
    xn = LayerNorm(x) * gamma + beta
    q,k,v = xn @ qkv (split)
    out = causal_softmax(q k^T / sqrt(K)) @ v @ o_proj + x

Sharding: 8 cores = 4 batches x 2 query-halves, folded-causal split into 4
classes of 256 query rows with key extents 512*(e+1) (identical program on
all cores; per-core behavior comes only from the permuted inputs/masks).

Each core receives ONLY its own 1024 tokens (8 chunks of 128): x as bf16
for the LN/projection path, plus an fp32 copy for the residual. k/v for the
peer's 1024 tokens arrive via per-half AllGathers (k and v separately; the
k gathers are the score-critical ones and trigger by mid-A0). All big
matmuls run in fp8 (e4m3) with DoubleRow perf mode. Weights are host-scaled
by 64 into fp8 range; scales are folded into PSUM evictions (q,k,v: 1/64;
out^T: 1/16; the softmax-denominator "ones" vector is 4.0 so recip =
1/(4*sums) exactly compensates oT/16 @ (64*wo) = 4 * out @ wo).

gamma is folded into the projection weights host-side; beta becomes bias
columns on the q/k evictions plus a (beta@Wv)@Wo correction folded into the
residual input. The x transpose runs on the vector engine (32x32 stream
transpose blocks), freeing the PE entirely for matmuls.

Queue discipline (critical for overlap):
  gpsimd = the 4 AllGather triggers + half-1 gather-output prefetch ONLY.
  scalar = ALL large input DMAs (hand-interleaved in consumption order) +
           PSUM evictions + v/k spills (dependency-paced).
  sync   = wo prefetch, half-0 gather loads, masks, recip reloads, resid.
  vector = LN stats/apply + x transpose, mask muls, D scale+add.
wq lives in its own pool that stays open until the B-phase tiles are
allocated, so no B tile can alias its SBUF (its readers run to the very end
of A0 and would otherwise stall B's prefetch DMAs).
"""
import os
import sys

import numpy as np

sys.path.insert(0, "/opt/trn_rl_repo")


def _install_ntff_hook():
    """Register the axon NTFF profile hook bass_utils expects (the image's
    antenv package lacks axon_hooks); degrades to no-op when unavailable."""
    import types
    if "antenv.axon_hooks" in sys.modules:
        return
    try:
        from trn_agent_boot.trn_boot import _ntff_profile_via_ctypes
        hook = _ntff_profile_via_ctypes("/opt/axon/libaxon_pjrt.so")
    except Exception:
        hook = None
    m = types.ModuleType("antenv.axon_hooks")
    m.get_axon_ntff_profile_hook = lambda: hook
    sys.modules["antenv.axon_hooks"] = m


_install_ntff_hook()

import ml_dtypes  # noqa: E402
import concourse.bass as bass  # noqa: E402
import concourse.tile as tile  # noqa: E402
from concourse import bacc, mybir  # noqa: E402
from concourse.bass_utils import run_bass_kernel_spmd  # noqa: E402

F32 = mybir.dt.float32
BF16 = mybir.dt.bfloat16
F8 = mybir.dt.float8e4
AF = mybir.ActivationFunctionType
OP = mybir.AluOpType
DR = mybir.MatmulPerfMode.DoubleRow

B, S, H, KEY = 4, 2048, 2048, 2048
NCHK = 16                 # 128-row hidden chunks (contraction)
NTOK = 8                  # own 128-token chunks per core
EPS = 1e-5
SCALE = 1.0 / float(np.sqrt(KEY))
WSCALE = 64.0             # host multiplier on weights before fp8 cast
ABASE = [0, 4, 12, 24]    # attn^T tile base index per class
ATOT = 40                 # total k-chunk tiles across classes


def build():
    nc = bacc.Bacc("TRN2", target_bir_lowering=False, debug=False, num_devices=8)

    x_d = nc.dram_tensor("x", [NTOK * 128, H], BF16, kind="ExternalInput")
    xr_d = nc.dram_tensor("xr", [NTOK * 128, H], F32, kind="ExternalInput")
    # weights pre-transposed on host for contiguous DMA:
    # wq/wk: [p, kc, hc, j]; wv: [kvt, p, hc, 512]; wo: [p, kvc, j] (fp8, x64)
    wq_d = nc.dram_tensor("wq", [128, NCHK, NCHK, 128], F8, kind="ExternalInput")
    wk_d = nc.dram_tensor("wk", [128, NCHK, NCHK, 128], F8, kind="ExternalInput")
    wv_d = nc.dram_tensor("wv", [4, 128, NCHK, 512], F8, kind="ExternalInput")
    wo_d = nc.dram_tensor("wo", [128, NCHK, H], F8, kind="ExternalInput")
    bqk_d = nc.dram_tensor("bqk", [2, 128, NCHK], F32, kind="ExternalInput")
    mask_d = nc.dram_tensor("mask", [4, 4, 128, 256], F8, kind="ExternalInput")
    y_d = nc.dram_tensor("y", [1024, H], F32, kind="ExternalOutput")
    DBG = bool(os.environ.get("K_DEBUG"))
    if DBG:
        dbg_s = nc.dram_tensor("dbg_s", [4, 256], F32, kind="ExternalOutput")
        dbg_q = nc.dram_tensor("dbg_q", [NCHK, 128, 1024], F8, kind="ExternalOutput")
        dbg_k = nc.dram_tensor("dbg_k", [2, 2, NCHK, 128, 512], F8,
                               kind="ExternalOutput")
        dbg_v = nc.dram_tensor("dbg_v", [2, 8, 128, KEY], F8, kind="ExternalOutput")
        dbg_a = nc.dram_tensor("dbg_a", [ATOT, 128, 256], F8, kind="ExternalOutput")
        dbg_o = nc.dram_tensor("dbg_o", [NCHK, 128, 1024], F8, kind="ExternalOutput")
        dbg_xn = nc.dram_tensor("dbg_xn", [NCHK, 128, 1024], F8,
                                kind="ExternalOutput")
    ssp_d = nc.dram_tensor("ssp", [4, 256], F32, kind="Internal")
    # split spill buffers per half: k^T tiles [4,128,2048] (ks[kc] at row
    # kc//4, cols (kc%4)*512) and v chunks [4,128,2048]; separate gathers so
    # the (score-critical) k exchange starts as early as possible
    ka_d = nc.dram_tensor("ka", [4, 128, KEY], F8, kind="Internal")
    kb_d = nc.dram_tensor("kb", [4, 128, KEY], F8, kind="Internal")
    va_d = nc.dram_tensor("va", [4, 128, KEY], F8, kind="Internal")
    vb_d = nc.dram_tensor("vb", [4, 128, KEY], F8, kind="Internal")
    kap_d = nc.dram_tensor("kap", [2, 4, 128, KEY], F8, kind="Internal")
    kbp_d = nc.dram_tensor("kbp", [2, 4, 128, KEY], F8, kind="Internal")
    vap_d = nc.dram_tensor("vap", [2, 4, 128, KEY], F8, kind="Internal")
    vbp_d = nc.dram_tensor("vbp", [2, 4, 128, KEY], F8, kind="Internal")
    GROUPS = [[2 * p, 2 * p + 1] for p in range(4)]

    ident = nc.inline_tensor(np.eye(128).astype(ml_dtypes.bfloat16),
                             name="ident")

    with tile.TileContext(nc) as tc:
        with (
            tc.tile_pool(name="small", bufs=1) as small,
            tc.tile_pool(name="p_main", bufs=1) as p_main,
        ):
            recip = small.tile([128, 8], F32)         # 1/(4*sums) per q-chunk
            bqcol = small.tile([128, NCHK], F32)      # beta@Wq bias, [p, kc]
            bkcol = small.tile([128, NCHK], F32)      # beta@Wk bias
            ones = small.tile([128, 1], F8)           # 4.0 (denominator scale)
            id8_sb = small.tile([128, 128], BF16)

            # persistent (never alias A0 pools -> gather-output loads carry no
            # SBUF WAR and cannot stall behind late A0 compute)
            xnT = p_main.tile([128, NCHK, 1024], F8)  # x_norm^T [hid_p, hc, tok]
            qT = p_main.tile([128, NCHK, 1024], F8)   # q^T [key_p, kc, tok]
            vts = p_main.tile([128, 16, KEY], F8)     # v [tok_p, slot(r*8+lc), kv]

            nc.vector.memset(ones[:], 4.0)

            # PE warm-up: dependency-free junk matmuls keep the PE-HAM busy
            # window active from ~7us so the first real matmuls run at 2.4GHz
            ju = small.tile([128, 2, 128], F8)
            nc.vector.memset(ju[:], 0.0)
            with tc.tile_pool(name="pp_j", bufs=1,
                              space=bass.MemorySpace.PSUM) as pp_j:
                psj = pp_j.tile([128, 128], F32)
                for _ in range(110):
                    nc.tensor.matmul(psj[:], ju[:], ju[:], start=True,
                                     stop=True, perf_mode=DR)

            # wq in its own pool, closed only after the B-phase tiles are
            # allocated: B tiles then cannot alias wq's SBUF space
            wqp0 = tc.tile_pool(name="wqp", bufs=1)
            wqp = wqp0.__enter__()
            wq_sb = wqp.tile([128, NCHK, NCHK, 128], F8)

            # ---------- A0: LN + DVE transpose + v + k + q ----------
            with (
                nc.named_scope("ln_transpose"),
                tc.tile_pool(name="a0x", bufs=6) as a0x,
                tc.tile_pool(name="a0x67", bufs=2) as a0x67,
                tc.tile_pool(name="a0xp", bufs=3) as a0xp,
                tc.tile_pool(name="a0s", bufs=4) as a0s,
                tc.tile_pool(name="wvp", bufs=1) as wvp,
                tc.tile_pool(name="vst", bufs=2) as vst,
                tc.tile_pool(name="kst", bufs=2) as kst,
                tc.tile_pool(name="pp_v", bufs=2, space=bass.MemorySpace.PSUM) as pp_v,
                tc.tile_pool(name="pp_k", bufs=2, space=bass.MemorySpace.PSUM) as pp_k,
                tc.tile_pool(name="pp_q", bufs=2, space=bass.MemorySpace.PSUM) as pp_q,
                tc.tile_pool(name="pp_tr", bufs=2,
                             space=bass.MemorySpace.PSUM) as pp_tr,
            ):
                # ALL large input DMAs ride the scalar queue, hand-interleaved
                # in consumption order (a second queue just steals bandwidth
                # from the critical stream)
                wk_sb = wvp.tile([128, NCHK, NCHK, 128], F8)   # [p, kc, hc, j]
                wv_sb = wvp.tile([128, 4, NCHK, 512], F8)
                nc.sync.dma_start(id8_sb[:], ident[:])
                nc.sync.dma_start(bqcol[:], bqk_d[0])
                nc.sync.dma_start(bkcol[:], bqk_d[1])
                xts = []

                def load_x(tci):
                    x_t = a0x.tile([128, H], BF16, tag="x")
                    nc.scalar.dma_start(x_t[:], x_d[tci * 128:(tci + 1) * 128, :])
                    xts.append(x_t)

                load_x(0)
                load_x(1)
                nc.scalar.dma_start(wv_sb[:, 0], wv_d[0])
                nc.scalar.dma_start(wv_sb[:, 1], wv_d[1])
                load_x(2)
                load_x(3)
                nc.scalar.dma_start(wv_sb[:, 2], wv_d[2])
                nc.scalar.dma_start(wv_sb[:, 3], wv_d[3])
                nc.scalar.dma_start(wk_sb[:, 0:8], wk_d[:, 0:8])
                nc.scalar.dma_start(wk_sb[:, 8:16], wk_d[:, 8:16])
                load_x(4)
                load_x(5)
                # x6/x7 ride sync from a dedicated pool: zero WAR waits, so
                # they land by ~30us and the chunk-6/7 LN stats never hold the
                # vector queue (whose emission order gates xp0-3)
                for tci in (6, 7):
                    x_t = a0x67.tile([128, H], BF16, tag="x67")
                    nc.sync.dma_start(x_t[:], x_d[tci * 128:(tci + 1) * 128, :])
                    xts.append(x_t)
                nc.scalar.dma_start(wq_sb[:], wq_d[:])

                xps = {}

                def ln_chunk(tci, dve):
                    """LN chunk tci; transpose into xnT either on the DVE
                    (16 32x32-block stream transposes, fp8 in/out) or later
                    on the PE (pe_transpose below)."""
                    x_t = xts[tci]
                    st = a0s.tile([128, 4, 6], F32, tag="st")
                    for j in range(4):
                        nc.vector.bn_stats(st[:, j, :], x_t[:, j * 512:(j + 1) * 512])
                    ag = a0s.tile([128, 2], F32, tag="ag")
                    nc.vector.bn_aggr(ag[:], st[:])
                    veps = a0s.tile([128, 1], F32, tag="veps")
                    nc.vector.tensor_scalar_add(veps[:], ag[:, 1:2], EPS)
                    sq = a0s.tile([128, 1], F32, tag="sq")
                    nc.scalar.sqrt(sq[:], veps[:])
                    rstd = a0s.tile([128, 1], F32, tag="rstd")
                    nc.vector.reciprocal(rstd[:], sq[:])
                    nmr = a0s.tile([128, 1], F32, tag="nmr")
                    nc.vector.tensor_scalar(nmr[:], ag[:, 0:1], rstd[:], -1.0,
                                            OP.mult, OP.mult)
                    xp = a0xp.tile([128, H], F8 if dve else BF16, tag="xp")
                    nc.vector.tensor_scalar(xp[:], x_t[:], rstd[:], nmr[:],
                                            OP.mult, OP.add)
                    xps[tci] = xp
                    if not dve:
                        return
                    xpb = xp[:].rearrange("p (hc j) -> p hc j", j=128)
                    for a in range(4):
                        for b in range(4):
                            nc.vector.transpose(
                                xnT[32 * a:32 * a + 32, :,
                                    tci * 128 + 32 * b:tci * 128 + 32 * b + 32],
                                xpb[32 * b:32 * b + 32, :, 32 * a:32 * a + 32])

                def pe_transpose(tci):
                    """PE-side transpose of chunk tci (4 psum groups)."""
                    xp = xps[tci]
                    for hg in range(4):
                        ps = pp_tr.tile([128, 512], BF16, tag="tr")
                        for hh in range(4):
                            hc = 4 * hg + hh
                            nc.tensor.transpose(
                                ps[:, hh * 128:(hh + 1) * 128],
                                xp[:, hc * 128:(hc + 1) * 128], id8_sb[:])
                        nc.scalar.copy(
                            xnT[:, 4 * hg:4 * hg + 4,
                                tci * 128:(tci + 1) * 128],
                            ps[:].rearrange("p (h j) -> p h j", j=128))

                def v_pass(lc):
                    """v for own local chunk lc -> spill row lc%4, spilled
                    per 512-col quarter (smaller SBUF + earlier spills)."""
                    vd = [va_d, vb_d][lc // 4]
                    for kvt in range(4):
                        ps = pp_v.tile([128, 512], F32, tag="v")
                        for hp in range(NCHK // 2):
                            nc.tensor.matmul(
                                ps[:],
                                xnT[:, 2 * hp:2 * hp + 2, lc * 128:(lc + 1) * 128],
                                wv_sb[:, kvt, 2 * hp:2 * hp + 2, :],
                                start=(hp == 0), stop=(hp == NCHK // 2 - 1),
                                perf_mode=DR)
                        vs = vst.tile([128, 512], F8, tag="vs")
                        nc.scalar.activation(vs[:], ps[:], AF.Identity,
                                             scale=1.0 / WSCALE)
                        nc.scalar.dma_start(
                            vd[lc % 4, :, kvt * 512:(kvt + 1) * 512], vs[:])

                def gather_v(half):
                    vd = [va_d, vb_d][half]
                    gout = [vap_d, vbp_d][half]
                    nc.gpsimd.collective_compute(
                        "AllGather", OP.bypass, replica_groups=GROUPS,
                        ins=[vd.ap().opt()], outs=[gout.ap().opt()])

                def k_pass(half):
                    """k^T for local tok chunks 4*half..4*half+3 (xnT cols
                    half*512..half*512+512); spills then gathers immediately
                    (scores depend only on k, so this is the critical CC)."""
                    kd = [ka_d, kb_d][half]
                    gout = [kap_d, kbp_d][half]
                    for kc in range(NCHK):
                        psk = pp_k.tile([128, 512], F32, tag="k")
                        for hp in range(NCHK // 2):
                            nc.tensor.matmul(
                                psk[:], wk_sb[:, kc, 2 * hp:2 * hp + 2, :],
                                xnT[:, 2 * hp:2 * hp + 2,
                                    half * 512:(half + 1) * 512],
                                start=(hp == 0), stop=(hp == NCHK // 2 - 1),
                                perf_mode=DR)
                        ks = kst.tile([128, 512], F8, tag="ks")
                        nc.scalar.activation(ks[:], psk[:], AF.Identity,
                                             scale=1.0 / WSCALE,
                                             bias=bkcol[:, kc:kc + 1])
                        nc.scalar.dma_start(
                            kd[kc // 4, :, (kc % 4) * 512:(kc % 4 + 1) * 512],
                            ks[:])
                    nc.gpsimd.collective_compute(
                        "AllGather", OP.bypass, replica_groups=GROUPS,
                        ins=[kd.ap().opt()], outs=[gout.ap().opt()])

                def q_pass(g):
                    """q^T for xnT cols g*512..(g+1)*512 (classes 2g, 2g+1)."""
                    for kc in range(NCHK):
                        psq = pp_q.tile([128, 512], F32, tag="q")
                        for hp in range(NCHK // 2):
                            nc.tensor.matmul(
                                psq[:], wq_sb[:, kc, 2 * hp:2 * hp + 2, :],
                                xnT[:, 2 * hp:2 * hp + 2, g * 512:(g + 1) * 512],
                                start=(hp == 0), stop=(hp == NCHK // 2 - 1),
                                perf_mode=DR)
                        nc.scalar.activation(qT[:, kc, g * 512:(g + 1) * 512],
                                             psq[:], AF.Identity,
                                             scale=1.0 / WSCALE,
                                             bias=bqcol[:, kc:kc + 1])

                for tci in range(4):
                    ln_chunk(tci, dve=False)
                for tci in range(4, NTOK):
                    ln_chunk(tci, dve=True)
                # PE transposes chunks 0-3 (quick start) while the DVE
                # transposes 4-7 underneath the v/k matmuls; k passes trigger
                # their gathers as early as their chunks allow
                for tci in range(4):
                    pe_transpose(tci)
                v_pass(0)
                v_pass(1)
                k_pass(0)
                v_pass(2)
                v_pass(3)
                gather_v(0)
                v_pass(4)
                v_pass(5)
                k_pass(1)
                v_pass(6)
                v_pass(7)
                gather_v(1)
                q_pass(0)
                q_pass(1)

            # ---------- pools for attn^T, out^T, o_proj weights ----------
            p_bc0 = tc.tile_pool(name="p_bc", bufs=1)
            p_bc = p_bc0.__enter__()
            wo_sb = p_bc.tile([128, NCHK, H], F8)     # o_proj weights; created
            # first so it lands on the earliest-dying A0 space (x/xp pools)
            oT = p_bc.tile([128, NCHK, 1024], F8)     # out^T [kv_p, kvc, q]
            # ktqs/aT die at the end of B: separate pool, closed before the
            # D pools open so the residual-prefetch tiles reuse their space
            p_ktqa0 = tc.tile_pool(name="p_ktqa", bufs=1)
            p_ktqa = p_ktqa0.__enter__()
            ktqs = [p_ktqa.tile([128, 4, 4, 512], F8, name=f"ktq{t}")
                    for t in range(4)]                # k^T tiles per (half, r)
            aT = p_ktqa.tile([128, ATOT, 256], F8)    # attn^T tiles

            # ---------- B+C interleaved: scores -> attn^T; out^T per class ----
            with (
                nc.named_scope("scores"),
                tc.tile_pool(name="bm", bufs=4) as bm,
                tc.tile_pool(name="bs", bufs=4) as bs,
                tc.tile_pool(name="pp_s", bufs=2, space=bass.MemorySpace.PSUM) as pp_s,
                tc.tile_pool(name="pp_o", bufs=2, space=bass.MemorySpace.PSUM) as pp_o,
                tc.tile_pool(name="pp_sum", bufs=4,
                             space=bass.MemorySpace.PSUM) as pp_sum,
            ):
                # sync is idle early: pull the o_proj weights in long before
                # use (its SBUF aliases the early-dying x/xp pools)
                nc.sync.dma_start(wo_sb[:], wo_d[:])
                # v resident: [tok_p, slot(r*8+lc), kv]
                for r in range(2):
                    nc.sync.dma_start(
                        vts[:, r * 8:r * 8 + 4, :],
                        vap_d[r].rearrange("lc p j -> p lc j"))
                # gpsimd (idle after the 4 collective triggers) prefetches the
                # half-1 gather outputs well before use
                for r in range(2):
                    nc.gpsimd.dma_start(
                        ktqs[2 + r][:],
                        kbp_d[r].rearrange("q p (k j) -> p q k j", j=512))
                for r in range(2):
                    nc.gpsimd.dma_start(
                        vts[:, r * 8 + 4:r * 8 + 8, :],
                        vbp_d[r].rearrange("lc p j -> p lc j"))

                ps_sums = [pp_sum.tile([1, 256], F32, tag="sum", name=f"psum{e}")
                           for e in range(4)]
                pending = []

                def emit_c(e):
                    """out^T class e: needs aT slots lc<=2e+1 (both ranks)."""
                    for kvc in range(NCHK):
                        ps_o = pp_o.tile([128, 256], F32, tag="o")
                        steps = [(r, j) for r in range(2) for j in range(e + 1)]
                        for si, (r, j) in enumerate(steps):
                            nc.tensor.matmul(
                                ps_o[:],
                                vts[:, r * 8 + 2 * j:r * 8 + 2 * j + 2,
                                    kvc * 128:(kvc + 1) * 128],
                                aT[:, ABASE[e] + r * 2 * (e + 1) + 2 * j:
                                   ABASE[e] + r * 2 * (e + 1) + 2 * j + 2, :],
                                start=(si == 0), stop=(si == len(steps) - 1),
                                perf_mode=DR)
                        nc.scalar.activation(oT[:, kvc, e * 256:(e + 1) * 256],
                                             ps_o[:], AF.Identity, scale=1.0 / 16)

                # slot (r, lc): rank r's local tok chunk lc; tile-grouped by
                # (half, r) with 4 lc each. Class e consumes lc < 2*(e+1).
                for half in range(2):
                    for r in range(2):
                        # k^T tiles packed [q][p][(k j)] with kc = q*4 + k
                        ktq4 = ktqs[2 * half + r]
                        if half == 0:
                            nc.sync.dma_start(
                                ktq4[:], kap_d[r].rearrange(
                                    "q p (k j) -> p q k j", j=512))
                        ktq = ktq4[:].rearrange("p q k j -> p (q k) j")
                        for lcc in range(4):
                            lc = half * 4 + lcc
                            this_round = []
                            for e in range(lc // 2, 4):
                                ps_s = pp_s.tile([128, 256], F32, tag="s")
                                for kp in range(NCHK // 2):
                                    nc.tensor.matmul(
                                        ps_s[:],
                                        ktq[:, 2 * kp:2 * kp + 2,
                                            lcc * 128:(lcc + 1) * 128],
                                        qT[:, 2 * kp:2 * kp + 2,
                                           e * 256:(e + 1) * 256],
                                        start=(kp == 0), stop=(kp == NCHK // 2 - 1),
                                        perf_mode=DR)
                                dst = aT[:, ABASE[e] + r * 2 * (e + 1) + lc, :]
                                if lc // 2 == e:
                                    tmp = bs.tile([128, 256], F8, tag="exps")
                                    nc.scalar.activation(tmp[:], ps_s[:], AF.Exp,
                                                         scale=SCALE)
                                    mt = bm.tile([128, 256], F8, tag="mask")
                                    nc.sync.dma_start(mt[:], mask_d[e, r * 2 + lc % 2])
                                    nc.vector.tensor_mul(dst, tmp[:], mt[:])
                                else:
                                    nc.scalar.activation(dst, ps_s[:], AF.Exp,
                                                         scale=SCALE)
                                this_round.append((e, (r, lc), dst))
                            for e, pos, src2 in pending:
                                nc.tensor.matmul(ps_sums[e][:], ones[:, 0:1], src2,
                                                 start=(pos == (0, 0)),
                                                 stop=(pos == (1, 2 * e + 1)))
                            pending = this_round
                            if r == 1 and lc % 2 == 1:
                                ecl = lc // 2    # class ecl complete
                                for e, pos, src2 in pending:
                                    nc.tensor.matmul(
                                        ps_sums[e][:], ones[:, 0:1], src2,
                                        start=(pos == (0, 0)),
                                        stop=(pos == (1, 2 * e + 1)))
                                pending = []
                                # class ecl's denominator is final: start the
                                # recip round-trip now so o_proj never waits
                                srow = bs.tile([1, 256], F32, tag="srow",
                                               name=f"srow{ecl}")
                                nc.scalar.copy(srow[:], ps_sums[ecl][:])
                                nc.scalar.dma_start(ssp_d[ecl], srow[:])
                                scol = bs.tile([128, 2], F32, tag="scol",
                                               name=f"scol{ecl}")
                                nc.sync.dma_start(
                                    scol[:],
                                    ssp_d[ecl].rearrange("(j p) -> p j", p=128))
                                nc.vector.reciprocal(
                                    recip[:, 2 * ecl:2 * ecl + 2], scol[:])
                                emit_c(ecl)

            if DBG:
                nc.sync.dma_start(dbg_s[:], ssp_d[:])
                for r in range(2):
                    for q in range(4):
                        nc.sync.dma_start(
                            dbg_k[0, r, q * 4:(q + 1) * 4],
                            kap_d[r, q].rearrange("p (k j) -> k p j", j=512))
                        nc.sync.dma_start(
                            dbg_k[1, r, q * 4:(q + 1) * 4],
                            kbp_d[r, q].rearrange("p (k j) -> k p j", j=512))
                    nc.sync.dma_start(dbg_v[r, 0:4], vap_d[r])
                    nc.sync.dma_start(dbg_v[r, 4:8], vbp_d[r])
                for kc in range(NCHK):
                    nc.sync.dma_start(dbg_q[kc], qT[:, kc, :])
                    nc.sync.dma_start(dbg_o[kc], oT[:, kc, :])
                    nc.sync.dma_start(dbg_xn[kc], xnT[:, kc, :])
                for t in range(ATOT):
                    nc.sync.dma_start(dbg_a[t], aT[:, t, :])

            p_ktqa0.__exit__(None, None, None)

            # ---------- D: y = diag(recip) (oT^T @ Wo) + x ----------
            with (
                nc.named_scope("o_proj"),
                tc.tile_pool(name="dx", bufs=16) as dx,
                tc.tile_pool(name="dy", bufs=6) as dy,
                tc.tile_pool(name="pp_y", bufs=8, space=bass.MemorySpace.PSUM) as pp_y,
            ):
                for ht in range(4):
                    for qg in range(2):
                        # residual prefetch ahead of the matmul group
                        xres = []
                        for i in range(4):
                            qc = qg * 4 + i
                            xt = dx.tile([128, 512], F32, tag="xr")
                            nc.sync.dma_start(xt[:],
                                              xr_d[qc * 128:(qc + 1) * 128,
                                                   ht * 512:(ht + 1) * 512])
                            xres.append(xt)
                        psy = [pp_y.tile([128, 512], F32, tag="y", name=f"psy{i}")
                               for i in range(4)]
                        for kp in range(NCHK // 2):
                            for i in range(4):
                                qc = qg * 4 + i
                                nc.tensor.matmul(
                                    psy[i][:],
                                    oT[:, 2 * kp:2 * kp + 2, qc * 128:(qc + 1) * 128],
                                    wo_sb[:, 2 * kp:2 * kp + 2,
                                          ht * 512:(ht + 1) * 512],
                                    start=(kp == 0), stop=(kp == NCHK // 2 - 1),
                                    perf_mode=DR)
                        for i in range(4):
                            qc = qg * 4 + i
                            ysb = dy.tile([128, 512], F32, tag="y")
                            nc.vector.scalar_tensor_tensor(
                                ysb[:], psy[i][:], recip[:, qc:qc + 1], xres[i][:],
                                OP.mult, OP.add)
                            eng = nc.scalar if i % 2 else nc.gpsimd
                            eng.dma_start(y_d[qc * 128:(qc + 1) * 128,
                                              ht * 512:(ht + 1) * 512],
                                          ysb[:])
            p_bc0.__exit__(None, None, None)
            wqp0.__exit__(None, None, None)
    nc.compile()
    return nc


_NC_CACHE = None


def _get_nc():
    global _NC_CACHE
    if _NC_CACHE is None:
        _NC_CACHE = build()
    return _NC_CACHE


def make_in_maps(x, qkv, o_proj, gamma, beta):
    qkv = np.asarray(qkv, dtype=np.float32)
    o_proj = np.asarray(o_proj, dtype=np.float32)
    gamma = np.asarray(gamma, dtype=np.float32)
    beta = np.asarray(beta, dtype=np.float32)
    F8NP = ml_dtypes.float8_e4m3
    BF16NP = ml_dtypes.bfloat16

    # gamma folds into the projection weights (row scaling); beta becomes
    # bias columns for q/k and a (beta@Wv)@Wo correction on the residual
    qkv_g = gamma[:, None] * qkv

    def prep_qk(w):  # [H, KEY] -> [p, kc, hc, j] fp8 (x64)
        t = (WSCALE * w).reshape(NCHK, 128, NCHK, 128)  # [hc, p, kc, j]
        return np.ascontiguousarray(t.transpose(1, 2, 0, 3)).astype(F8NP)

    def prep_v(w):  # [H, KEY] -> [kvt, p, hc, 512] fp8 (x64)
        t = (WSCALE * w).reshape(NCHK, 128, 4, 512)     # [hc, p, kvt, j]
        return np.ascontiguousarray(t.transpose(2, 1, 0, 3)).astype(F8NP)

    def prep_o(w):  # [KEY, H] -> [p, kvc, j] fp8 (x64)
        t = (WSCALE * w).reshape(NCHK, 128, H)          # [kvc, p, j]
        return np.ascontiguousarray(t.transpose(1, 0, 2)).astype(F8NP)

    wq8 = prep_qk(qkv_g[:, :KEY])
    wk8 = prep_qk(qkv_g[:, KEY:2 * KEY])
    wv8 = prep_v(qkv_g[:, 2 * KEY:])
    wo8 = prep_o(o_proj)
    bq = beta @ qkv[:, :KEY]
    bk = beta @ qkv[:, KEY:2 * KEY]
    bv = beta @ qkv[:, 2 * KEY:]
    yv = bv @ o_proj                                    # residual correction
    bqk = np.ascontiguousarray(np.stack(
        [bq.reshape(NCHK, 128).T, bk.reshape(NCHK, 128).T])).astype(np.float32)
    in_maps, metas = [], []
    for c in range(8):
        b, h = c // 2, c % 2
        own = [4 * e + 2 * h + i for e in range(4) for i in (0, 1)]
        ti = np.concatenate([np.arange(gc * 128, gc * 128 + 128) for gc in own])
        x_own = np.ascontiguousarray(x[b][ti], dtype=np.float32)
        # mask[e][2r+j]: k tok-slot (rank r, quad e, j) holds true chunk
        # 4e+2r+j; q col c of class e is true row ti[256e+c].
        mask = np.zeros((4, 4, 128, 256), dtype=F8NP)
        for e in range(4):
            qp = ti[256 * e:256 * e + 256]
            for r in range(2):
                for j in range(2):
                    kp = (4 * e + 2 * r + j) * 128 + np.arange(128)
                    mask[e, 2 * r + j] = (kp[:, None] <= qp[None, :]).astype(F8NP)
        in_maps.append({"x": x_own.astype(BF16NP), "xr": x_own + yv[None, :],
                        "wq": wq8, "wk": wk8, "wv": wv8, "wo": wo8,
                        "bqk": bqk, "mask": mask})
        metas.append((b, ti))
    return in_maps, metas


def gather(results, metas, dtype):
    out = np.empty((B, S, H), dtype=dtype)
    for c, (b, ti) in enumerate(metas):
        out[b][ti] = results[c]["y"]
    return out


def kernel(x, qkv, o_proj, gamma, beta, _trace=False):
    x = np.asarray(x, dtype=np.float32)
    nc = _get_nc()
    in_maps, metas = make_in_maps(x, qkv, o_proj, gamma, beta)
    res = run_bass_kernel_spmd(nc, in_maps, core_ids=list(range(8)), trace=_trace)
    out = gather(res.results, metas, np.float32)
    if _trace:
        kernel.last_result = res
    return out


# revision 26
# speedup vs baseline: 1.0258x; 1.0258x over previous
"""Trainium2 Bass kernel: pre-LN single-head causal attention + residual.

Reference computation (B=4, S=2048, H=K=2048, fp32):
    xn = LayerNorm(x) * gamma + beta
    q,k,v = xn @ qkv (split)
    out = causal_softmax(q k^T / sqrt(K)) @ v @ o_proj + x

Sharding: 8 cores = 4 batches x 2 query-halves, folded-causal split into 4
classes of 256 query rows with key extents 512*(e+1) (identical program on
all cores; per-core behavior comes only from the permuted inputs/masks).

Each core receives ONLY its own 1024 tokens (8 chunks of 128): x as bf16
for the LN/projection path, plus an fp32 copy for the residual. k/v for the
peer's 1024 tokens arrive via per-half AllGathers (k and v separately; the
k gathers are the score-critical ones and trigger by mid-A0). All big
matmuls run in fp8 (e4m3) with DoubleRow perf mode. Weights are host-scaled
by 64 into fp8 range; scales are folded into PSUM evictions (q,k,v: 1/64;
out^T: 1/16; the softmax-denominator "ones" vector is 4.0 so recip =
1/(4*sums) exactly compensates oT/16 @ (64*wo) = 4 * out @ wo).

gamma is folded into the projection weights host-side; beta becomes bias
columns on the q/k evictions plus a (beta@Wv)@Wo correction folded into the
residual input. The x transpose runs on the vector engine (32x32 stream
transpose blocks), freeing the PE entirely for matmuls.

Queue discipline (critical for overlap):
  gpsimd = the 4 AllGather triggers + half-1 gather-output prefetch ONLY.
  scalar = ALL large input DMAs (hand-interleaved in consumption order) +
           PSUM evictions + v/k spills (dependency-paced).
  sync   = wo prefetch, half-0 gather loads, masks, recip reloads, resid.
  vector = LN stats/apply + x transpose, mask muls, D scale+add.
wq lives in its own pool that stays open until the B-phase tiles are
allocated, so no B tile can alias its SBUF (its readers run to the very end
of A0 and would otherwise stall B's prefetch DMAs).
"""
import os
import sys

import numpy as np

sys.path.insert(0, "/opt/trn_rl_repo")


def _install_ntff_hook():
    """Register the axon NTFF profile hook bass_utils expects (the image's
    antenv package lacks axon_hooks); degrades to no-op when unavailable."""
    import types
    if "antenv.axon_hooks" in sys.modules:
        return
    try:
        from trn_agent_boot.trn_boot import _ntff_profile_via_ctypes
        hook = _ntff_profile_via_ctypes("/opt/axon/libaxon_pjrt.so")
    except Exception:
        hook = None
    m = types.ModuleType("antenv.axon_hooks")
    m.get_axon_ntff_profile_hook = lambda: hook
    sys.modules["antenv.axon_hooks"] = m


_install_ntff_hook()

import ml_dtypes  # noqa: E402
import concourse.bass as bass  # noqa: E402
import concourse.tile as tile  # noqa: E402
from concourse import bacc, mybir  # noqa: E402
from concourse.bass_utils import run_bass_kernel_spmd  # noqa: E402

F32 = mybir.dt.float32
BF16 = mybir.dt.bfloat16
F8 = mybir.dt.float8e4
AF = mybir.ActivationFunctionType
OP = mybir.AluOpType
DR = mybir.MatmulPerfMode.DoubleRow

B, S, H, KEY = 4, 2048, 2048, 2048
NCHK = 16                 # 128-row hidden chunks (contraction)
NTOK = 8                  # own 128-token chunks per core
EPS = 1e-5
SCALE = 1.0 / float(np.sqrt(KEY))
WSCALE = 64.0             # host multiplier on weights before fp8 cast
ABASE = [0, 4, 12, 24]    # attn^T tile base index per class
ATOT = 40                 # total k-chunk tiles across classes


def build():
    nc = bacc.Bacc("TRN2", target_bir_lowering=False, debug=False, num_devices=8)

    x_d = nc.dram_tensor("x", [NTOK * 128, H], BF16, kind="ExternalInput")
    xr_d = nc.dram_tensor("xr", [NTOK * 128, H], F32, kind="ExternalInput")
    # weights pre-transposed on host for contiguous DMA:
    # wq/wk: [p, kc, hc, j]; wv: [kvt, p, hc, 512]; wo: [p, kvc, j] (fp8, x64)
    wq_d = nc.dram_tensor("wq", [128, NCHK, NCHK, 128], F8, kind="ExternalInput")
    wk_d = nc.dram_tensor("wk", [128, NCHK, NCHK, 128], F8, kind="ExternalInput")
    wv_d = nc.dram_tensor("wv", [4, 128, NCHK, 512], F8, kind="ExternalInput")
    wo_d = nc.dram_tensor("wo", [128, NCHK, H], F8, kind="ExternalInput")
    bqk_d = nc.dram_tensor("bqk", [2, 128, NCHK], F32, kind="ExternalInput")
    mask_d = nc.dram_tensor("mask", [4, 4, 128, 256], F8, kind="ExternalInput")
    y_d = nc.dram_tensor("y", [1024, H], F32, kind="ExternalOutput")
    DBG = bool(os.environ.get("K_DEBUG"))
    if DBG:
        dbg_s = nc.dram_tensor("dbg_s", [4, 256], F32, kind="ExternalOutput")
        dbg_q = nc.dram_tensor("dbg_q", [NCHK, 128, 1024], F8, kind="ExternalOutput")
        dbg_k = nc.dram_tensor("dbg_k", [2, 2, NCHK, 128, 512], F8,
                               kind="ExternalOutput")
        dbg_v = nc.dram_tensor("dbg_v", [2, 8, 128, KEY], F8, kind="ExternalOutput")
        dbg_a = nc.dram_tensor("dbg_a", [ATOT, 128, 256], F8, kind="ExternalOutput")
        dbg_o = nc.dram_tensor("dbg_o", [NCHK, 128, 1024], F8, kind="ExternalOutput")
        dbg_xn = nc.dram_tensor("dbg_xn", [NCHK, 128, 1024], F8,
                                kind="ExternalOutput")
    ssp_d = nc.dram_tensor("ssp", [4, 256], F32, kind="Internal")
    # split spill buffers per half: k^T tiles [4,128,2048] (ks[kc] at row
    # kc//4, cols (kc%4)*512) and v chunks [4,128,2048]; separate gathers so
    # the (score-critical) k exchange starts as early as possible
    ka_d = nc.dram_tensor("ka", [4, 128, KEY], F8, kind="Internal")
    kb_d = nc.dram_tensor("kb", [4, 128, KEY], F8, kind="Internal")
    va_d = nc.dram_tensor("va", [4, 128, KEY], F8, kind="Internal")
    vb_d = nc.dram_tensor("vb", [4, 128, KEY], F8, kind="Internal")
    kap_d = nc.dram_tensor("kap", [2, 4, 128, KEY], F8, kind="Internal")
    kbp_d = nc.dram_tensor("kbp", [2, 4, 128, KEY], F8, kind="Internal")
    vap_d = nc.dram_tensor("vap", [2, 4, 128, KEY], F8, kind="Internal")
    vbp_d = nc.dram_tensor("vbp", [2, 4, 128, KEY], F8, kind="Internal")
    GROUPS = [[2 * p, 2 * p + 1] for p in range(4)]

    ident = nc.inline_tensor(np.eye(128).astype(ml_dtypes.bfloat16),
                             name="ident")

    with tile.TileContext(nc) as tc:
        with (
            tc.tile_pool(name="small", bufs=1) as small,
            tc.tile_pool(name="p_main", bufs=1) as p_main,
        ):
            recip = small.tile([128, 8], F32)         # 1/(4*sums) per q-chunk
            bqcol = small.tile([128, NCHK], F32)      # beta@Wq bias, [p, kc]
            bkcol = small.tile([128, NCHK], F32)      # beta@Wk bias
            ones = small.tile([128, 1], F8)           # 4.0 (denominator scale)
            id8_sb = small.tile([128, 128], BF16)

            # persistent (never alias A0 pools -> gather-output loads carry no
            # SBUF WAR and cannot stall behind late A0 compute)
            xnT = p_main.tile([128, NCHK, 1024], F8)  # x_norm^T [hid_p, hc, tok]
            qT = p_main.tile([128, NCHK, 1024], F8)   # q^T [key_p, kc, tok]
            vts = p_main.tile([128, 16, KEY], F8)     # v [tok_p, slot(r*8+lc), kv]

            nc.vector.memset(ones[:], 4.0)

            # PE warm-up: dependency-free junk matmuls keep the PE-HAM busy
            # window active from ~7us so the first real matmuls run at 2.4GHz
            ju = small.tile([128, 2, 128], F8)
            nc.vector.memset(ju[:], 0.0)
            with tc.tile_pool(name="pp_j", bufs=1,
                              space=bass.MemorySpace.PSUM) as pp_j:
                psj = pp_j.tile([128, 128], F32)
                for _ in range(110):
                    nc.tensor.matmul(psj[:], ju[:], ju[:], start=True,
                                     stop=True, perf_mode=DR)

            # wq in its own pool, closed only after the B-phase tiles are
            # allocated: B tiles then cannot alias wq's SBUF space
            wqp0 = tc.tile_pool(name="wqp", bufs=1)
            wqp = wqp0.__enter__()
            wq_sb = wqp.tile([128, NCHK, NCHK, 128], F8)

            # ---------- A0: LN + DVE transpose + v + k + q ----------
            with (
                nc.named_scope("ln_transpose"),
                tc.tile_pool(name="a0x", bufs=6) as a0x,
                tc.tile_pool(name="a0xp", bufs=3) as a0xp,
                tc.tile_pool(name="a0s", bufs=4) as a0s,
                tc.tile_pool(name="wvp", bufs=1) as wvp,
                tc.tile_pool(name="vst", bufs=1) as vst,
                tc.tile_pool(name="kst", bufs=2) as kst,
                tc.tile_pool(name="pp_v", bufs=2, space=bass.MemorySpace.PSUM) as pp_v,
                tc.tile_pool(name="pp_k", bufs=2, space=bass.MemorySpace.PSUM) as pp_k,
                tc.tile_pool(name="pp_q", bufs=2, space=bass.MemorySpace.PSUM) as pp_q,
                tc.tile_pool(name="pp_tr", bufs=2,
                             space=bass.MemorySpace.PSUM) as pp_tr,
            ):
                # ALL large input DMAs ride the scalar queue, hand-interleaved
                # in consumption order (a second queue just steals bandwidth
                # from the critical stream)
                wk_sb = wvp.tile([128, NCHK, NCHK, 128], F8)   # [p, kc, hc, j]
                wv_sb = wvp.tile([128, 4, NCHK, 512], F8)
                nc.sync.dma_start(id8_sb[:], ident[:])
                nc.sync.dma_start(bqcol[:], bqk_d[0])
                nc.sync.dma_start(bkcol[:], bqk_d[1])
                xts = []

                def load_x(tci):
                    x_t = a0x.tile([128, H], BF16, tag="x")
                    nc.scalar.dma_start(x_t[:], x_d[tci * 128:(tci + 1) * 128, :])
                    xts.append(x_t)

                load_x(0)
                load_x(1)
                nc.scalar.dma_start(wv_sb[:, 0], wv_d[0])
                nc.scalar.dma_start(wv_sb[:, 1], wv_d[1])
                load_x(2)
                load_x(3)
                nc.scalar.dma_start(wv_sb[:, 2], wv_d[2])
                nc.scalar.dma_start(wv_sb[:, 3], wv_d[3])
                nc.scalar.dma_start(wk_sb[:, 0:8], wk_d[:, 0:8])
                nc.scalar.dma_start(wk_sb[:, 8:16], wk_d[:, 8:16])
                load_x(4)
                load_x(5)
                # x6/x7 ride sync: their WAR waits (on chunk-0/1 consumers)
                # must not sit in front of the LN sqrts on the scalar FIFO
                for tci in (6, 7):
                    x_t = a0x.tile([128, H], BF16, tag="x")
                    nc.sync.dma_start(x_t[:], x_d[tci * 128:(tci + 1) * 128, :])
                    xts.append(x_t)
                nc.scalar.dma_start(wq_sb[:], wq_d[:])

                xps = {}

                def ln_chunk(tci, dve):
                    """LN chunk tci; transpose into xnT either on the DVE
                    (16 32x32-block stream transposes, fp8 in/out) or later
                    on the PE (pe_transpose below)."""
                    x_t = xts[tci]
                    st = a0s.tile([128, 4, 6], F32, tag="st")
                    for j in range(4):
                        nc.vector.bn_stats(st[:, j, :], x_t[:, j * 512:(j + 1) * 512])
                    ag = a0s.tile([128, 2], F32, tag="ag")
                    nc.vector.bn_aggr(ag[:], st[:])
                    veps = a0s.tile([128, 1], F32, tag="veps")
                    nc.vector.tensor_scalar_add(veps[:], ag[:, 1:2], EPS)
                    sq = a0s.tile([128, 1], F32, tag="sq")
                    nc.scalar.sqrt(sq[:], veps[:])
                    rstd = a0s.tile([128, 1], F32, tag="rstd")
                    nc.vector.reciprocal(rstd[:], sq[:])
                    nmr = a0s.tile([128, 1], F32, tag="nmr")
                    nc.vector.tensor_scalar(nmr[:], ag[:, 0:1], rstd[:], -1.0,
                                            OP.mult, OP.mult)
                    xp = a0xp.tile([128, H], F8 if dve else BF16, tag="xp")
                    nc.vector.tensor_scalar(xp[:], x_t[:], rstd[:], nmr[:],
                                            OP.mult, OP.add)
                    xps[tci] = xp
                    if not dve:
                        return
                    xpb = xp[:].rearrange("p (hc j) -> p hc j", j=128)
                    for a in range(4):
                        for b in range(4):
                            nc.vector.transpose(
                                xnT[32 * a:32 * a + 32, :,
                                    tci * 128 + 32 * b:tci * 128 + 32 * b + 32],
                                xpb[32 * b:32 * b + 32, :, 32 * a:32 * a + 32])

                def pe_transpose(tci):
                    """PE-side transpose of chunk tci (4 psum groups)."""
                    xp = xps[tci]
                    for hg in range(4):
                        ps = pp_tr.tile([128, 512], BF16, tag="tr")
                        for hh in range(4):
                            hc = 4 * hg + hh
                            nc.tensor.transpose(
                                ps[:, hh * 128:(hh + 1) * 128],
                                xp[:, hc * 128:(hc + 1) * 128], id8_sb[:])
                        nc.scalar.copy(
                            xnT[:, 4 * hg:4 * hg + 4,
                                tci * 128:(tci + 1) * 128],
                            ps[:].rearrange("p (h j) -> p h j", j=128))

                def v_pass(lc):
                    """v for own local chunk lc -> spill row lc%4."""
                    vs = vst.tile([128, KEY], F8, tag="vs")
                    for kvt in range(4):
                        ps = pp_v.tile([128, 512], F32, tag="v")
                        for hp in range(NCHK // 2):
                            nc.tensor.matmul(
                                ps[:],
                                xnT[:, 2 * hp:2 * hp + 2, lc * 128:(lc + 1) * 128],
                                wv_sb[:, kvt, 2 * hp:2 * hp + 2, :],
                                start=(hp == 0), stop=(hp == NCHK // 2 - 1),
                                perf_mode=DR)
                        nc.scalar.activation(vs[:, kvt * 512:(kvt + 1) * 512],
                                             ps[:], AF.Identity,
                                             scale=1.0 / WSCALE)
                    vd = [va_d, vb_d][lc // 4]
                    nc.scalar.dma_start(vd[lc % 4][:], vs[:])

                def gather_v(half):
                    vd = [va_d, vb_d][half]
                    gout = [vap_d, vbp_d][half]
                    nc.gpsimd.collective_compute(
                        "AllGather", OP.bypass, replica_groups=GROUPS,
                        ins=[vd.ap().opt()], outs=[gout.ap().opt()])

                def k_pass(half):
                    """k^T for local tok chunks 4*half..4*half+3 (xnT cols
                    half*512..half*512+512); spills then gathers immediately
                    (scores depend only on k, so this is the critical CC)."""
                    kd = [ka_d, kb_d][half]
                    gout = [kap_d, kbp_d][half]
                    for kc in range(NCHK):
                        psk = pp_k.tile([128, 512], F32, tag="k")
                        for hp in range(NCHK // 2):
                            nc.tensor.matmul(
                                psk[:], wk_sb[:, kc, 2 * hp:2 * hp + 2, :],
                                xnT[:, 2 * hp:2 * hp + 2,
                                    half * 512:(half + 1) * 512],
                                start=(hp == 0), stop=(hp == NCHK // 2 - 1),
                                perf_mode=DR)
                        ks = kst.tile([128, 512], F8, tag="ks")
                        nc.scalar.activation(ks[:], psk[:], AF.Identity,
                                             scale=1.0 / WSCALE,
                                             bias=bkcol[:, kc:kc + 1])
                        nc.scalar.dma_start(
                            kd[kc // 4, :, (kc % 4) * 512:(kc % 4 + 1) * 512],
                            ks[:])
                    nc.gpsimd.collective_compute(
                        "AllGather", OP.bypass, replica_groups=GROUPS,
                        ins=[kd.ap().opt()], outs=[gout.ap().opt()])

                def q_pass(g):
                    """q^T for xnT cols g*512..(g+1)*512 (classes 2g, 2g+1)."""
                    for kc in range(NCHK):
                        psq = pp_q.tile([128, 512], F32, tag="q")
                        for hp in range(NCHK // 2):
                            nc.tensor.matmul(
                                psq[:], wq_sb[:, kc, 2 * hp:2 * hp + 2, :],
                                xnT[:, 2 * hp:2 * hp + 2, g * 512:(g + 1) * 512],
                                start=(hp == 0), stop=(hp == NCHK // 2 - 1),
                                perf_mode=DR)
                        nc.scalar.activation(qT[:, kc, g * 512:(g + 1) * 512],
                                             psq[:], AF.Identity,
                                             scale=1.0 / WSCALE,
                                             bias=bqcol[:, kc:kc + 1])

                for tci in range(4):
                    ln_chunk(tci, dve=False)
                for tci in range(4, NTOK):
                    ln_chunk(tci, dve=True)
                # PE transposes chunks 0-3 (quick start) while the DVE
                # transposes 4-7 underneath the v/k matmuls; k passes trigger
                # their gathers as early as their chunks allow
                for tci in range(4):
                    pe_transpose(tci)
                v_pass(0)
                v_pass(1)
                k_pass(0)
                v_pass(2)
                v_pass(3)
                gather_v(0)
                v_pass(4)
                v_pass(5)
                k_pass(1)
                v_pass(6)
                v_pass(7)
                gather_v(1)
                q_pass(0)
                q_pass(1)

            # ---------- pools for attn^T, out^T, o_proj weights ----------
            p_bc0 = tc.tile_pool(name="p_bc", bufs=1)
            p_bc = p_bc0.__enter__()
            wo_sb = p_bc.tile([128, NCHK, H], F8)     # o_proj weights; created
            # first so it lands on the earliest-dying A0 space (x/xp pools)
            oT = p_bc.tile([128, NCHK, 1024], F8)     # out^T [kv_p, kvc, q]
            # ktqs/aT die at the end of B: separate pool, closed before the
            # D pools open so the residual-prefetch tiles reuse their space
            p_ktqa0 = tc.tile_pool(name="p_ktqa", bufs=1)
            p_ktqa = p_ktqa0.__enter__()
            ktqs = [p_ktqa.tile([128, 4, 4, 512], F8, name=f"ktq{t}")
                    for t in range(4)]                # k^T tiles per (half, r)
            aT = p_ktqa.tile([128, ATOT, 256], F8)    # attn^T tiles

            # ---------- B+C interleaved: scores -> attn^T; out^T per class ----
            with (
                nc.named_scope("scores"),
                tc.tile_pool(name="bm", bufs=4) as bm,
                tc.tile_pool(name="bs", bufs=4) as bs,
                tc.tile_pool(name="pp_s", bufs=2, space=bass.MemorySpace.PSUM) as pp_s,
                tc.tile_pool(name="pp_o", bufs=2, space=bass.MemorySpace.PSUM) as pp_o,
                tc.tile_pool(name="pp_sum", bufs=4,
                             space=bass.MemorySpace.PSUM) as pp_sum,
            ):
                # sync is idle early: pull the o_proj weights in long before
                # use (its SBUF aliases the early-dying x/xp pools)
                nc.sync.dma_start(wo_sb[:], wo_d[:])
                # v resident: [tok_p, slot(r*8+lc), kv]
                for r in range(2):
                    nc.sync.dma_start(
                        vts[:, r * 8:r * 8 + 4, :],
                        vap_d[r].rearrange("lc p j -> p lc j"))
                # gpsimd (idle after the 4 collective triggers) prefetches the
                # half-1 gather outputs well before use
                for r in range(2):
                    nc.gpsimd.dma_start(
                        ktqs[2 + r][:],
                        kbp_d[r].rearrange("q p (k j) -> p q k j", j=512))
                for r in range(2):
                    nc.gpsimd.dma_start(
                        vts[:, r * 8 + 4:r * 8 + 8, :],
                        vbp_d[r].rearrange("lc p j -> p lc j"))

                ps_sums = [pp_sum.tile([1, 256], F32, tag="sum", name=f"psum{e}")
                           for e in range(4)]
                pending = []

                def emit_c(e):
                    """out^T class e: needs aT slots lc<=2e+1 (both ranks)."""
                    for kvc in range(NCHK):
                        ps_o = pp_o.tile([128, 256], F32, tag="o")
                        steps = [(r, j) for r in range(2) for j in range(e + 1)]
                        for si, (r, j) in enumerate(steps):
                            nc.tensor.matmul(
                                ps_o[:],
                                vts[:, r * 8 + 2 * j:r * 8 + 2 * j + 2,
                                    kvc * 128:(kvc + 1) * 128],
                                aT[:, ABASE[e] + r * 2 * (e + 1) + 2 * j:
                                   ABASE[e] + r * 2 * (e + 1) + 2 * j + 2, :],
                                start=(si == 0), stop=(si == len(steps) - 1),
                                perf_mode=DR)
                        nc.scalar.activation(oT[:, kvc, e * 256:(e + 1) * 256],
                                             ps_o[:], AF.Identity, scale=1.0 / 16)

                # slot (r, lc): rank r's local tok chunk lc; tile-grouped by
                # (half, r) with 4 lc each. Class e consumes lc < 2*(e+1).
                for half in range(2):
                    for r in range(2):
                        # k^T tiles packed [q][p][(k j)] with kc = q*4 + k
                        ktq4 = ktqs[2 * half + r]
                        if half == 0:
                            nc.sync.dma_start(
                                ktq4[:], kap_d[r].rearrange(
                                    "q p (k j) -> p q k j", j=512))
                        ktq = ktq4[:].rearrange("p q k j -> p (q k) j")
                        for lcc in range(4):
                            lc = half * 4 + lcc
                            this_round = []
                            for e in range(lc // 2, 4):
                                ps_s = pp_s.tile([128, 256], F32, tag="s")
                                for kp in range(NCHK // 2):
                                    nc.tensor.matmul(
                                        ps_s[:],
                                        ktq[:, 2 * kp:2 * kp + 2,
                                            lcc * 128:(lcc + 1) * 128],
                                        qT[:, 2 * kp:2 * kp + 2,
                                           e * 256:(e + 1) * 256],
                                        start=(kp == 0), stop=(kp == NCHK // 2 - 1),
                                        perf_mode=DR)
                                dst = aT[:, ABASE[e] + r * 2 * (e + 1) + lc, :]
                                if lc // 2 == e:
                                    tmp = bs.tile([128, 256], F8, tag="exps")
                                    nc.scalar.activation(tmp[:], ps_s[:], AF.Exp,
                                                         scale=SCALE)
                                    mt = bm.tile([128, 256], F8, tag="mask")
                                    nc.sync.dma_start(mt[:], mask_d[e, r * 2 + lc % 2])
                                    nc.vector.tensor_mul(dst, tmp[:], mt[:])
                                else:
                                    nc.scalar.activation(dst, ps_s[:], AF.Exp,
                                                         scale=SCALE)
                                this_round.append((e, (r, lc), dst))
                            for e, pos, src2 in pending:
                                nc.tensor.matmul(ps_sums[e][:], ones[:, 0:1], src2,
                                                 start=(pos == (0, 0)),
                                                 stop=(pos == (1, 2 * e + 1)))
                            pending = this_round
                            if r == 1 and lc % 2 == 1:
                                ecl = lc // 2    # class ecl complete
                                for e, pos, src2 in pending:
                                    nc.tensor.matmul(
                                        ps_sums[e][:], ones[:, 0:1], src2,
                                        start=(pos == (0, 0)),
                                        stop=(pos == (1, 2 * e + 1)))
                                pending = []
                                # class ecl's denominator is final: start the
                                # recip round-trip now so o_proj never waits
                                srow = bs.tile([1, 256], F32, tag="srow",
                                               name=f"srow{ecl}")
                                nc.scalar.copy(srow[:], ps_sums[ecl][:])
                                nc.scalar.dma_start(ssp_d[ecl], srow[:])
                                scol = bs.tile([128, 2], F32, tag="scol",
                                               name=f"scol{ecl}")
                                nc.sync.dma_start(
                                    scol[:],
                                    ssp_d[ecl].rearrange("(j p) -> p j", p=128))
                                nc.vector.reciprocal(
                                    recip[:, 2 * ecl:2 * ecl + 2], scol[:])
                                emit_c(ecl)

            if DBG:
                nc.sync.dma_start(dbg_s[:], ssp_d[:])
                for r in range(2):
                    for q in range(4):
                        nc.sync.dma_start(
                            dbg_k[0, r, q * 4:(q + 1) * 4],
                            kap_d[r, q].rearrange("p (k j) -> k p j", j=512))
                        nc.sync.dma_start(
                            dbg_k[1, r, q * 4:(q + 1) * 4],
                            kbp_d[r, q].rearrange("p (k j) -> k p j", j=512))
                    nc.sync.dma_start(dbg_v[r, 0:4], vap_d[r])
                    nc.sync.dma_start(dbg_v[r, 4:8], vbp_d[r])
                for kc in range(NCHK):
                    nc.sync.dma_start(dbg_q[kc], qT[:, kc, :])
                    nc.sync.dma_start(dbg_o[kc], oT[:, kc, :])
                    nc.sync.dma_start(dbg_xn[kc], xnT[:, kc, :])
                for t in range(ATOT):
                    nc.sync.dma_start(dbg_a[t], aT[:, t, :])

            p_ktqa0.__exit__(None, None, None)

            # ---------- D: y = diag(recip) (oT^T @ Wo) + x ----------
            with (
                nc.named_scope("o_proj"),
                tc.tile_pool(name="dx", bufs=16) as dx,
                tc.tile_pool(name="dy", bufs=6) as dy,
                tc.tile_pool(name="pp_y", bufs=8, space=bass.MemorySpace.PSUM) as pp_y,
            ):
                for ht in range(4):
                    for qg in range(2):
                        # residual prefetch ahead of the matmul group
                        xres = []
                        for i in range(4):
                            qc = qg * 4 + i
                            xt = dx.tile([128, 512], F32, tag="xr")
                            nc.sync.dma_start(xt[:],
                                              xr_d[qc * 128:(qc + 1) * 128,
                                                   ht * 512:(ht + 1) * 512])
                            xres.append(xt)
                        psy = [pp_y.tile([128, 512], F32, tag="y", name=f"psy{i}")
                               for i in range(4)]
                        for kp in range(NCHK // 2):
                            for i in range(4):
                                qc = qg * 4 + i
                                nc.tensor.matmul(
                                    psy[i][:],
                                    oT[:, 2 * kp:2 * kp + 2, qc * 128:(qc + 1) * 128],
                                    wo_sb[:, 2 * kp:2 * kp + 2,
                                          ht * 512:(ht + 1) * 512],
                                    start=(kp == 0), stop=(kp == NCHK // 2 - 1),
                                    perf_mode=DR)
                        for i in range(4):
                            qc = qg * 4 + i
                            ysb = dy.tile([128, 512], F32, tag="y")
                            nc.vector.scalar_tensor_tensor(
                                ysb[:], psy[i][:], recip[:, qc:qc + 1], xres[i][:],
                                OP.mult, OP.add)
                            eng = nc.scalar if i % 2 else nc.gpsimd
                            eng.dma_start(y_d[qc * 128:(qc + 1) * 128,
                                              ht * 512:(ht + 1) * 512],
                                          ysb[:])
            p_bc0.__exit__(None, None, None)
            wqp0.__exit__(None, None, None)
    nc.compile()
    return nc


_NC_CACHE = None


def _get_nc():
    global _NC_CACHE
    if _NC_CACHE is None:
        _NC_CACHE = build()
    return _NC_CACHE


def make_in_maps(x, qkv, o_proj, gamma, beta):
    qkv = np.asarray(qkv, dtype=np.float32)
    o_proj = np.asarray(o_proj, dtype=np.float32)
    gamma = np.asarray(gamma, dtype=np.float32)
    beta = np.asarray(beta, dtype=np.float32)
    F8NP = ml_dtypes.float8_e4m3
    BF16NP = ml_dtypes.bfloat16

    # gamma folds into the projection weights (row scaling); beta becomes
    # bias columns for q/k and a (beta@Wv)@Wo correction on the residual
    qkv_g = gamma[:, None] * qkv

    def prep_qk(w):  # [H, KEY] -> [p, kc, hc, j] fp8 (x64)
        t = (WSCALE * w).reshape(NCHK, 128, NCHK, 128)  # [hc, p, kc, j]
        return np.ascontiguousarray(t.transpose(1, 2, 0, 3)).astype(F8NP)

    def prep_v(w):  # [H, KEY] -> [kvt, p, hc, 512] fp8 (x64)
        t = (WSCALE * w).reshape(NCHK, 128, 4, 512)     # [hc, p, kvt, j]
        return np.ascontiguousarray(t.transpose(2, 1, 0, 3)).astype(F8NP)

    def prep_o(w):  # [KEY, H] -> [p, kvc, j] fp8 (x64)
        t = (WSCALE * w).reshape(NCHK, 128, H)          # [kvc, p, j]
        return np.ascontiguousarray(t.transpose(1, 0, 2)).astype(F8NP)

    wq8 = prep_qk(qkv_g[:, :KEY])
    wk8 = prep_qk(qkv_g[:, KEY:2 * KEY])
    wv8 = prep_v(qkv_g[:, 2 * KEY:])
    wo8 = prep_o(o_proj)
    bq = beta @ qkv[:, :KEY]
    bk = beta @ qkv[:, KEY:2 * KEY]
    bv = beta @ qkv[:, 2 * KEY:]
    yv = bv @ o_proj                                    # residual correction
    bqk = np.ascontiguousarray(np.stack(
        [bq.reshape(NCHK, 128).T, bk.reshape(NCHK, 128).T])).astype(np.float32)
    in_maps, metas = [], []
    for c in range(8):
        b, h = c // 2, c % 2
        own = [4 * e + 2 * h + i for e in range(4) for i in (0, 1)]
        ti = np.concatenate([np.arange(gc * 128, gc * 128 + 128) for gc in own])
        x_own = np.ascontiguousarray(x[b][ti], dtype=np.float32)
        # mask[e][2r+j]: k tok-slot (rank r, quad e, j) holds true chunk
        # 4e+2r+j; q col c of class e is true row ti[256e+c].
        mask = np.zeros((4, 4, 128, 256), dtype=F8NP)
        for e in range(4):
            qp = ti[256 * e:256 * e + 256]
            for r in range(2):
                for j in range(2):
                    kp = (4 * e + 2 * r + j) * 128 + np.arange(128)
                    mask[e, 2 * r + j] = (kp[:, None] <= qp[None, :]).astype(F8NP)
        in_maps.append({"x": x_own.astype(BF16NP), "xr": x_own + yv[None, :],
                        "wq": wq8, "wk": wk8, "wv": wv8, "wo": wo8,
                        "bqk": bqk, "mask": mask})
        metas.append((b, ti))
    return in_maps, metas


def gather(results, metas, dtype):
    out = np.empty((B, S, H), dtype=dtype)
    for c, (b, ti) in enumerate(metas):
        out[b][ti] = results[c]["y"]
    return out


def kernel(x, qkv, o_proj, gamma, beta, _trace=False):
    x = np.asarray(x, dtype=np.float32)
    nc = _get_nc()
    in_maps, metas = make_in_maps(x, qkv, o_proj, gamma, beta)
    res = run_bass_kernel_spmd(nc, in_maps, core_ids=list(range(8)), trace=_trace)
    out = gather(res.results, metas, np.float32)
    if _trace:
        kernel.last_result = res
    return out
